# revision 5
# baseline (speedup 1.0000x reference)
"""GCN (2-layer GraphConv) Trainium2 kernel, 8-core SPMD.

Math: reference computes out = relu(A @ (relu(A @ (X W1)) W2)) with
A[r,c] = sum of vals over edges (r,c).  Dense matmul commutes with the
SpMM (spmm(X @ W) == spmm(X) @ W), so each layer is computed as
  z = spmm(table); h = relu(z @ W)
which keeps the 128x128 weight matmuls on the core-local 12500-row
shard instead of the full 100k-node table.

Per layer, per core (rows sharded 12500/core):
  - edges are grouped host-side by (owner core, col-chunk of 25000)
    so gather indices fit int16.
  - HW dma_scatter_add loses updates for duplicate indices within one
    call (measured), but sequential calls accumulate exactly.  So each
    row's t-th in-chunk occurrence goes to a different 2048-token block
    and blocks are padded with distinct unused rows at val=0.
  - dma_gather 2048-token blocks from the DRAM table (512B/row),
  - per-128-token-slot val multiply (DVE tensor_scalar + ACT share),
  - dma_scatter_add into SBUF z accumulators; blocks alternate between
    two independent accumulator sets to halve the serial WAW chain,
  - z = set0 + set1, then PE: transpose z tile, matmul with W, ReLU on
    PSUM eviction, DMA out.

Layer 1 runs with table=X/w=W1, layer 2 with table=h1/w=W2 on the same
compiled NEFF; the halo exchange between layers is a host gather of the
8 h1 shards.
"""

import numpy as np
from contextlib import ExitStack

import concourse.bass as bass
import concourse.tile as tile
from concourse import bacc, mybir
from concourse.bass_utils import run_bass_kernel_spmd

# -------- geometry (hardcoded for the graded problem) --------
N_NODES = 100000
D = 128
NCORES = 8
NCHUNKS = 4
TOK_BLOCK = 1024
NZSETS = 2

ROWS_PER_CORE = N_NODES // NCORES            # 12500
NBLOCKS = (ROWS_PER_CORE + 127) // 128       # 98 row blocks of 128
R_PAD = NBLOCKS * 128                        # 12544
NGROUPS = (NBLOCKS + 1) // 2                 # 49 parity groups
CHUNK = -(-N_NODES // NCHUNKS)               # 25000 (< int16 max)

LAST_EXEC_NS = None


# ---------------------------------------------------------------------------
# host-side edge preprocessing
# ---------------------------------------------------------------------------

def _group_tokens(rows, cols, vals, rows_per_core, nchunks, chunk, ncores):
    core = rows // rows_per_core
    ch = cols // chunk
    gid = core * nchunks + ch
    order = np.argsort(gid, kind="stable")
    rows, cols, vals, gid = rows[order], cols[order], vals[order], gid[order]
    bounds = np.searchsorted(gid, np.arange(ncores * nchunks + 1))
    out = []
    for g in range(ncores * nchunks):
        s, e = bounds[g], bounds[g + 1]
        k, c = divmod(g, nchunks)
        out.append(((rows[s:e] - k * rows_per_core).astype(np.int64),
                    (cols[s:e] - c * chunk).astype(np.int64),
                    vals[s:e]))
    return out


def _block_assign(r_l, nblk):
    """occurrence-round-robin block id per token; requires multiplicity<=nblk."""
    order = np.argsort(r_l, kind="stable")
    r_s = r_l[order]
    n = len(r_s)
    if n == 0:
        return order, np.zeros(0, np.int64), 0
    newseg = np.r_[True, r_s[1:] != r_s[:-1]]
    seg_start = np.nonzero(newseg)[0]
    occ = np.arange(n) - np.repeat(seg_start, np.diff(np.r_[seg_start, n]))
    maxmult = int(occ.max()) + 1
    blk = (occ + r_s % nblk) % nblk
    return order, blk, maxmult


def prep_edges(adj_rows, adj_cols, adj_vals, rows_per_core=ROWS_PER_CORE,
               nchunks=NCHUNKS, chunk=CHUNK, tok_block=TOK_BLOCK,
               ncores=NCORES):
    """Returns (E_blk, per_core) with per-call-unique rows.

    per_core[k]: colidx/rowidx [nchunks,128,E_blk//16] i16 (lane-replicated
    x8), vals [nchunks,128,E_blk//128] f32 (token-order layout).
    """
    rows = np.asarray(adj_rows).astype(np.int64)
    cols = np.asarray(adj_cols).astype(np.int64)
    vals = np.asarray(adj_vals).astype(np.float32)
    groups = _group_tokens(rows, cols, vals, rows_per_core, nchunks, chunk,
                           ncores)

    nblk = max(2, -(-max(len(g[0]) for g in groups) // tok_block))
    # find nblk so every block load fits and multiplicity fits
    while True:
        ok = True
        assigns = []
        for r_l, c_l, v in groups:
            order, blk, maxmult = _block_assign(r_l, nblk)
            if maxmult > nblk or (len(blk) and
                                  np.bincount(blk, minlength=nblk).max() > tok_block):
                ok = False
                break
            assigns.append((order, blk))
        if ok:
            break
        nblk += 1

    E_blk = nblk * tok_block
    L = E_blk // 16

    per_core = []
    for k in range(ncores):
        colidx = np.zeros((nchunks, 16, L), np.int16)
        rowidx = np.zeros((nchunks, 16, L), np.int16)
        vtok = np.zeros((nchunks, E_blk), np.float32)
        for c in range(nchunks):
            r_l, c_l, v = groups[k * nchunks + c]
            order, blk = assigns[k * nchunks + c]
            r_l, c_l, v = r_l[order], c_l[order], v[order]
            bord = np.argsort(blk, kind="stable")
            boff = np.searchsorted(blk[bord], np.arange(nblk + 1))
            rstream = np.zeros(E_blk, np.int64)
            cstream = np.zeros(E_blk, np.int64)
            vstream = np.zeros(E_blk, np.float32)
            for b in range(nblk):
                s, e = boff[b], boff[b + 1]
                n = e - s
                base = b * tok_block
                sel = bord[s:e]
                rstream[base:base + n] = r_l[sel]
                cstream[base:base + n] = c_l[sel]
                vstream[base:base + n] = v[sel]
                npad = tok_block - n
                if npad:
                    used = np.zeros(rows_per_core, bool)
                    used[r_l[sel]] = True
                    filler = np.nonzero(~used)[0][:npad]
                    assert len(filler) == npad
                    rstream[base + n:base + tok_block] = filler
                    # cstream stays 0, vstream stays 0 -> adds exact 0
            colidx[c] = cstream.reshape(L, 16).T
            rowidx[c] = rstream.reshape(L, 16).T
            vtok[c] = vstream
        vtile = vtok.reshape(nchunks, E_blk // 128, 128).transpose(0, 2, 1)
        per_core.append(dict(
            colidx=np.tile(colidx, (1, 8, 1)).astype(np.int16),
            rowidx=np.tile(rowidx, (1, 8, 1)).astype(np.int16),
            vals=np.ascontiguousarray(vtile),
        ))
    return E_blk, per_core


# ---------------------------------------------------------------------------
# device kernel
# ---------------------------------------------------------------------------

def build_kernel(E_blk, n_nodes=N_NODES, nchunks=NCHUNKS, chunk=CHUNK,
                 nblocks=NBLOCKS, tok_block=TOK_BLOCK, nzsets=NZSETS,
                 nqueues=2, scratch=65536):
    dt = mybir.dt
    r_pad = nblocks * 128
    ngroups = (nblocks + 1) // 2
    nblk = E_blk // tok_block
    spb = tok_block // 128      # 128-token slots per block
    ipb = tok_block // 16       # idx columns per block

    nc = bacc.Bacc("TRN2", target_bir_lowering=False, debug=False,
                   num_devices=NCORES, num_swdge_queues=nqueues,
                   dynamic_dma_scratch_size=scratch)
    table = nc.dram_tensor("table", [n_nodes, D], dt.float32,
                           kind="ExternalInput")
    w = nc.dram_tensor("w", [D, D], dt.float32, kind="ExternalInput")
    colidx = nc.dram_tensor("colidx", [nchunks, 128, E_blk // 16], dt.int16,
                            kind="ExternalInput")
    rowidx = nc.dram_tensor("rowidx", [nchunks, 128, E_blk // 16], dt.int16,
                            kind="ExternalInput")
    vals = nc.dram_tensor("vals", [nchunks, 128, E_blk // 128], dt.float32,
                          kind="ExternalInput")
    hout = nc.dram_tensor("hout", [r_pad, D], dt.float32,
                          kind="ExternalOutput")
    ident = nc.inline_tensor(np.eye(128, dtype=np.float32), "ident")

    with tile.TileContext(nc) as tc, ExitStack() as ctx:
        zpool = ctx.enter_context(tc.tile_pool(name="z", bufs=1))
        msgpool = ctx.enter_context(tc.tile_pool(name="msg", bufs=4))
        cixpool = ctx.enter_context(tc.tile_pool(name="cix", bufs=2))
        rixpool = ctx.enter_context(tc.tile_pool(name="rix", bufs=2))
        vpool = ctx.enter_context(tc.tile_pool(name="v", bufs=2))
        cpool = ctx.enter_context(tc.tile_pool(name="consts", bufs=1))
        ztpool = ctx.enter_context(tc.tile_pool(name="zt", bufs=3))
        opool = ctx.enter_context(tc.tile_pool(name="o", bufs=3))
        pspool = ctx.enter_context(
            tc.tile_pool(name="ps", bufs=2, space=bass.MemorySpace.PSUM))

        wt = cpool.tile([128, 128], dt.float32)
        nc.sync.dma_start(wt[:], w[:])
        idt = cpool.tile([128, 128], dt.float32)
        nc.sync.dma_start(idt[:], ident[:])

        zs = []
        for s in range(nzsets):
            zA = zpool.tile([128, ngroups, 128], dt.float32, tag=f"zA{s}")
            zB = zpool.tile([128, ngroups, 128], dt.float32, tag=f"zB{s}")
            nc.vector.memset(zA[:], 0.0)
            nc.vector.memset(zB[:], 0.0)
            zs.append((zA, zB))

        for c in range(nchunks):
            ci = cixpool.tile([128, E_blk // 16], dt.int16)
            nc.sync.dma_start(ci[:], colidx[c])
            ri = rixpool.tile([128, E_blk // 16], dt.int16)
            nc.sync.dma_start(ri[:], rowidx[c])
            vv = vpool.tile([128, E_blk // 128], dt.float32)
            nc.sync.dma_start(vv[:], vals[c])
            tbl = table[c * chunk:(c + 1) * chunk, :]
            for b in range(nblk):
                msg = msgpool.tile([128, spb, 128], dt.float32)
                nc.gpsimd.dma_gather(
                    msg[:], tbl, ci[:, b * ipb:(b + 1) * ipb],
                    tok_block, tok_block, D, elem_step=D,
                    queue_num=0)
                for j in range(spb):
                    sv = vv[:, b * spb + j: b * spb + j + 1]
                    if j % 3 == 2:
                        nc.scalar.mul(msg[:, j, :], msg[:, j, :], sv)
                    else:
                        nc.vector.tensor_scalar_mul(msg[:, j, :], msg[:, j, :], sv)
                zA, zB = zs[(c * nblk + b) % nzsets]
                nc.gpsimd.dma_scatter_add(
                    zA[:], msg[:], ri[:, b * ipb:(b + 1) * ipb],
                    tok_block, tok_block, D,
                    sbuf_tokens_per_rank=128, parity_reg=0,
                    out_ap_other=zB[:], queue_num=min(1, nqueues - 1))

        # combine accumulator sets in place into set 0
        for s in range(1, nzsets):
            nc.vector.tensor_add(zs[0][0][:], zs[0][0][:], zs[s][0][:])
            nc.vector.tensor_add(zs[0][1][:], zs[0][1][:], zs[s][1][:])
        zA, zB = zs[0]

        for blk in range(nblocks):
            g, par = blk >> 1, blk & 1
            zsrc = zB if par else zA
            tp = pspool.tile([128, 128], dt.float32)
            nc.tensor.transpose(tp[:], zsrc[:, g, :], idt[:])
            zt = ztpool.tile([128, 128], dt.float32)
            nc.vector.tensor_copy(zt[:], tp[:])
            yp = pspool.tile([128, 128], dt.float32)
            nc.tensor.matmul(yp[:], zt[:], wt[:], start=True, stop=True)
            ho = opool.tile([128, 128], dt.float32)
            nc.scalar.activation(ho[:], yp[:],
                                 mybir.ActivationFunctionType.Relu)
            nc.sync.dma_start(hout[blk * 128:(blk + 1) * 128, :], ho[:])

    nc.compile()
    return nc


_NC_CACHE = {}


def _get_nc(E_blk):
    if E_blk not in _NC_CACHE:
        _NC_CACHE[E_blk] = build_kernel(E_blk)
    return _NC_CACHE[E_blk]


def _run_layer(nc, table_full, wmat, per_core, trace=False):
    in_maps = [
        dict(table=np.ascontiguousarray(table_full, dtype=np.float32),
             w=np.ascontiguousarray(wmat, dtype=np.float32),
             colidx=pc["colidx"], rowidx=pc["rowidx"], vals=pc["vals"])
        for pc in per_core
    ]
    res = run_bass_kernel_spmd(nc, in_maps, list(range(NCORES)), trace=trace)
    h = np.concatenate(
        [res.results[k]["hout"][:ROWS_PER_CORE] for k in range(NCORES)], axis=0)
    return h, res


def kernel(X_mask, adj_rows, adj_cols, adj_vals, W1, W2):
    global LAST_EXEC_NS
    E_blk, per_core = prep_edges(adj_rows, adj_cols, adj_vals)
    nc = _get_nc(E_blk)

    h1, res1 = _run_layer(nc, X_mask, W1, per_core)
    out, res2 = _run_layer(nc, h1, W2, per_core)

    ns = [r.exec_time_ns for r in (res1, res2)]
    LAST_EXEC_NS = sum(n for n in ns if n) if any(ns) else None
    return out.astype(np.float32)
